# revision 39
# baseline (speedup 1.0000x reference)
"""Trainium2 Bass kernel for the ExemplarModel (Mahalanobis-kNN attention).

Reference math (N=1024 queries, M=50000 exemplars, D=512, C=10 classes):
    dist[n,m]  = sum_d Sigma_inv[d] * (x[n,d] - e[m,d])^2
    att[n,m]   = exp(-beta * dist[n,m])
    logits[n,c]= segment_sum(att over exemplars with label c)
    out        = softmax(gamma * logits, axis=1)

Distribution: exemplars/labels sharded along M across 8 NeuronCores
(6250 each, zero-padded to 6272 = 49*128); x, Sigma_inv, beta replicated.
Each core computes partial per-class logits
    P[c,n] = sum_m onehot[m,c] * exp(2*beta*cross[n,m] - beta*e_sq[m])
with cross[m,n] = sum_d e[m,d] * (x*Sigma_inv)[n,d].

v2 design (evidence from the v1 trace: PE ~53us of matmul at 216ns/MM,
ACT ~53us of Exp at 1089ns/tile, ~30us of stalls from the on-device
e_sq pipeline and mid-run DMA waits):
  - e_sq bias is precomputed on the HOST (exact f32) and DMA'd as a
    [128, 49] tile: the entire f32 exemplar stream (12.8MB/core), the
    DVE square/reduce chain, and its semaphores are gone.
  - everything (fp8 eT, fp8 xsT, fp8 one-hot, bias) is preloaded to
    SBUF with a handful of large DMAs on the sync/gpsimd queues; the
    Scalar queue carries nothing but the 49 Exp instructions.
  - cross matmuls: fp8 DoubleRow, K=512 as 2 pair-chunks, free dim 512
    (2 halves of N), PSUM triple-buffered [128,1024] tiles.
  - segment matmuls: one-hot lhsT [128,16], batched 3 tiles at a time
    onto distinct 32-column strips via tile_position=(0, 32g) so the
    three 16-row matmuls run concurrently in the PE array; partial
    logits accumulate in one [96, 1024] PSUM region over all 49 tiles
    (strip g = t%3); the host sums the 3 strips (and the 8 cores).
  - warmup: dummy DoubleRow matmuls + a dummy Exp issued behind the
    first DMA so the PE HAM un-throttles and the ACT exp table loads
    while the exemplar preload streams.
The host combines: logits[n,c] = exp(-beta*x_sq[n]) * sum_cores P, then
gamma + softmax on the tiny [1024,10] result.
"""

import numpy as np
import ml_dtypes

import concourse.bass as bass
import concourse.bacc as bacc
import concourse.tile as tile
from concourse import mybir
from concourse import bass_utils

# Problem constants (hardcoded per contract; kernel.py must be self-contained).
N = 1024          # queries
M = 50000         # exemplars (global)
D = 512           # feature dim
C = 10            # classes
N_CORES = 8
M_LOC = M // N_CORES          # 6250 exemplars per core
P = 128                       # partitions
T_TILES = (M_LOC + P - 1) // P  # 49 tiles per core
M_PAD = T_TILES * P           # 6272
KC = D // P                   # 4 contraction chunks
CP = 16                       # one-hot pitch
NH = N // 512                 # matmul free-dim halves
OUT_ROWS = CP                 # partial-logit rows per core
ET_TILE_CUTS = [0, 3, 15, 31, 49]   # eT preload chunk boundaries (tiles)
C0_TILES = ET_TILE_CUTS[1]          # eT tiles merged into the c0 DMA

FP32 = mybir.dt.float32
FP8 = mybir.dt.float8e4
NP_FP8 = ml_dtypes.float8_e4m3


def build_nc(t_tiles=T_TILES, n=N, debug=False):
    """Build the per-core Bass program (SPMD: same program, per-core data)."""
    nc = bacc.Bacc("TRN2", target_bir_lowering=False, debug=debug,
                   num_devices=N_CORES)
    m_pad = t_tiles * P
    nh = n // 512

    # host pre-packs every tensor in its exact SBUF layout so each DMA is
    # one contiguous descriptor per partition (512 strided descriptors per
    # transfer cost ~2us of ring time each in v4)
    t_cuts = ET_TILE_CUTS
    # c0 = xsT + the first C0_TILES eT tiles, split into k01/k23 halves so
    # the j=0 cross matmuls can start while the second half streams
    c0_dram = [nc.dram_tensor(f"c0{half}", [P, 2 * (n + C0_TILES * P)], FP8,
                              kind="ExternalInput") for half in range(2)]
    eT_drams = [
        nc.dram_tensor(f"eT{c}", [P, KC * (t_cuts[c + 1] - t_cuts[c]) * P],
                       FP8, kind="ExternalInput")
        for c in range(1, len(t_cuts) - 1)
    ]
    w_dram = nc.dram_tensor("w", [P, t_tiles * CP], FP8, kind="ExternalInput")
    bt_dram = nc.dram_tensor("bt", [P, t_tiles + 1], FP32,
                             kind="ExternalInput")
    out_dram = nc.dram_tensor("out", [OUT_ROWS, n], FP32, kind="ExternalOutput")

    with tile.TileContext(nc) as tc:
        with (
            tc.tile_pool(name="const", bufs=1) as const_pool,
            tc.tile_pool(name="crossp", bufs=3, space="PSUM") as cross_pool,
            tc.tile_pool(name="logitp", bufs=1, space="PSUM") as logit_pool,
        ):
            # ---- preamble: preload everything; every DMA is contiguous
            # per partition. sync queue: c0 (xsT + first eT tiles in one
            # transfer) then the remaining eT chunks; scalar queue: the
            # tiny one-hot + bias/scale tensors (land first, and the
            # one-hot doubles as warmup-matmul fodder).
            # warmup matmuls on a memset junk tile: no DMA dependency, so
            # the PE HAM un-throttles while the preload is still in flight
            junk = const_pool.tile([P, 512], FP8, tag="junk")
            nc.gpsimd.memset(junk[:], 0)
            junk_3d = junk[:].rearrange("p (k m) -> p k m", m=256)
            warm_ps = cross_pool.tile([P, n], FP32, tag="cross")
            for wi in range(10):
                nc.tensor.matmul(
                    warm_ps[:, (wi % 2) * 512:(wi % 2) * 512 + 2 * P],
                    lhsT=junk_3d[:, :, 0:P],
                    rhs=junk_3d[:, :, 0:2 * P],
                    start=True, stop=True,
                    perf_mode=mybir.MatmulPerfMode.DoubleRow,
                    skip_group_check=True)

            c0_len = n + C0_TILES * P
            c0_sb = const_pool.tile([P, KC * c0_len], FP8, tag="c0")
            c0_3d = c0_sb[:].rearrange("p (k m) -> p k m", m=c0_len)
            xsT_3d = c0_3d[:, :, 0:n]
            nc.sync.dma_start(c0_sb[:, 0:2 * c0_len], c0_dram[0][:])

            bt_sb = const_pool.tile([P, t_tiles + 1], FP32, tag="bt")
            nc.sync.dma_start(bt_sb[:], bt_dram[:])
            twob = bt_sb[:, t_tiles:t_tiles + 1]

            nc.sync.dma_start(c0_sb[:, 2 * c0_len:], c0_dram[1][:])

            eT_sbs = [c0_3d[:, :, n:]]
            w_sb = const_pool.tile([P, t_tiles * CP], FP8, tag="w")
            for c, eTd in enumerate(eT_drams):
                mlen = (t_cuts[c + 2] - t_cuts[c + 1]) * P
                sb = const_pool.tile([P, KC * mlen], FP8, tag=f"eT{c}")
                eT_sbs.append(sb[:].rearrange("p (k m) -> p k m", m=mlen))
                nc.sync.dma_start(sb[:], eTd[:])
                if c == 0:  # one-hot is first needed by the t=4 seg matmul
                    nc.sync.dma_start(w_sb[:], w_dram[:])

            def eT_lhsT(t, j):
                c = next(ci for ci in range(len(t_cuts) - 1)
                         if t < t_cuts[ci + 1])
                mo = (t - t_cuts[c]) * P
                return eT_sbs[c][:, 2 * j:2 * j + 2, mo:mo + P]

            # one logits tile + one output-staging tile per n-half so the
            # drain of one half never false-serializes against the other
            logits_ps = [logit_pool.tile([OUT_ROWS, 512], FP32, name=f"lg{h}")
                         for h in range(nh)]
            out_sbs = [const_pool.tile([OUT_ROWS, 512], FP32, name=f"out{h}")
                       for h in range(nh)]

            # ---- main loop over exemplar tiles ----
            # every tile gets its own slot in one big att buffer (no reuse,
            # so Exp and segment matmuls each carry a single semaphore wait);
            # a DoubleRow segment matmul consumes two tiles at once, issued
            # two pairs behind so the PE never waits on ScalarE.
            w_pairs = w_sb[:].rearrange("p (t c) -> p t c", c=CP)
            att_all = const_pool.tile([P, t_tiles * n], FP8, tag="att")
            att_3d = att_all[:].rearrange("p (t n) -> p t n", n=n)
            n_pairs = t_tiles // 2

            def seg_pair(q, h):
                """DoubleRow segment matmul for tile pair q, n-half h."""
                nc.tensor.matmul(
                    logits_ps[h][:],
                    lhsT=w_pairs[:, 2 * q:2 * q + 2, :],
                    rhs=att_3d[:, 2 * q:2 * q + 2, h * 512:(h + 1) * 512],
                    start=(q == 0), stop=False,
                    perf_mode=mybir.MatmulPerfMode.DoubleRow,
                    skip_group_check=True)

            for t in range(t_tiles):
                # cross[m, n] = sum_d e[m,d] * xs[n,d]; fp8 DoubleRow,
                # K=512 as 2 pair-chunks, free dim 512 (2 halves of N)
                cross_ps = cross_pool.tile([P, n], FP32, tag="cross")
                for j in range(KC // 2):
                    for h in range(nh):
                        nc.tensor.matmul(
                            cross_ps[:, h * 512:(h + 1) * 512],
                            lhsT=eT_lhsT(t, j),
                            rhs=xsT_3d[:, 2 * j:2 * j + 2,
                                       h * 512:(h + 1) * 512],
                            start=(j == 0), stop=(j == KC // 2 - 1),
                            perf_mode=mybir.MatmulPerfMode.DoubleRow)

                # segment matmuls for the pair finished two tiles ago
                if t % 2 == 0 and t >= 4:
                    for h in range(nh):
                        seg_pair(t // 2 - 2, h)

                # att = exp(2*beta*cross - beta*e_sq)  (ACT reads PSUM)
                nc.scalar.activation(att_3d[:, t, :], cross_ps[:],
                                     mybir.ActivationFunctionType.Exp,
                                     bias=bt_sb[:, t:t + 1],
                                     scale=twob)

            # ---- drain + epilogue, pipelined per n-half: finish each
            # half's segment matmuls, copy that half of the PSUM logits to
            # SBUF on the idle DVE, DMA it out while the other half drains.
            last = t_tiles - 1
            for h in range(nh):
                seg_pair(n_pairs - 1, h)
                nc.tensor.matmul(
                    logits_ps[h][:],
                    lhsT=w_sb[:, last * CP:(last + 1) * CP],
                    rhs=att_3d[:, last, h * 512:(h + 1) * 512],
                    start=False, stop=True,
                    skip_group_check=True)
                # h0 copy on the idle DVE, h1 on ScalarE (free after the
                # last Exp) so the two drains run concurrently
                if h == 0:
                    nc.vector.tensor_scalar_mul(out_sbs[h][:],
                                                logits_ps[h][:], 1.0)
                else:
                    nc.scalar.copy(out_sbs[h][:], logits_ps[h][:])
                nc.sync.dma_start(out_dram[:, h * 512:(h + 1) * 512],
                                  out_sbs[h][:])

    nc.compile()
    return nc


def make_in_maps(x, exemplars, labels, Sigma_inv, beta, gamma,
                 t_tiles=T_TILES):
    """Shard the full inputs into per-core in_maps (host-side glue)."""
    x = np.asarray(x, dtype=np.float32)
    exemplars = np.asarray(exemplars, dtype=np.float32)
    labels = np.asarray(labels).astype(np.int64)
    Sigma_inv = np.asarray(Sigma_inv, dtype=np.float32)
    beta = float(np.asarray(beta).reshape(-1)[0])

    m_pad = t_tiles * P
    # xsT packed [P, KC, N]: partition p holds feature rows {p, 128+p, ...}
    xsT = (x * Sigma_inv).T.astype(NP_FP8)                # [D, N]
    xsT_packed = np.ascontiguousarray(
        xsT.reshape(KC, P, N).transpose(1, 0, 2)).reshape(P, KC * N)

    m_loc = M // N_CORES
    in_maps = []
    for c in range(N_CORES):
        e_shard = np.zeros((m_pad, D), dtype=np.float32)
        e_shard[:m_loc] = exemplars[c * m_loc:(c + 1) * m_loc]
        eT_shard = e_shard.T.astype(NP_FP8)               # [D, m_pad]
        eT_kpm = eT_shard.reshape(KC, P, m_pad).transpose(1, 0, 2)  # [P,KC,m]
        im = {}
        # c0 = [xsT | eT tiles 0..C0_TILES-1] interleaved per k-chunk,
        # split into k01 / k23 halves
        c0 = np.concatenate(
            [xsT_packed.reshape(P, KC, N), eT_kpm[:, :, :C0_TILES * P]],
            axis=2)
        im["c00"] = np.ascontiguousarray(c0[:, 0:2]).reshape(P, -1)
        im["c01"] = np.ascontiguousarray(c0[:, 2:4]).reshape(P, -1)
        for ci in range(1, len(ET_TILE_CUTS) - 1):
            a, b = ET_TILE_CUTS[ci] * P, ET_TILE_CUTS[ci + 1] * P
            im[f"eT{ci}"] = np.ascontiguousarray(
                eT_kpm[:, :, a:b]).reshape(P, KC * (b - a))
        # exact f32 per-exemplar bias -beta*e_sq[m] as [128, 49], plus the
        # activation scale 2*beta in the last column
        e_sq = np.einsum("md,md,d->m", e_shard, e_shard, Sigma_inv)
        bt = np.empty((P, t_tiles + 1), dtype=np.float32)
        bt[:, :t_tiles] = (-beta * e_sq).reshape(t_tiles, P).T
        bt[:, t_tiles] = 2.0 * beta
        im["bt"] = bt
        lab = labels[c * m_loc:(c + 1) * m_loc]
        onehot = np.zeros((m_pad, CP), dtype=np.float32)
        onehot[np.arange(m_loc), lab] = 1.0
        im["w"] = np.ascontiguousarray(
            onehot.reshape(t_tiles, P, CP).transpose(1, 0, 2)
            .reshape(P, t_tiles * CP)).astype(NP_FP8)
        in_maps.append(im)
    return in_maps


def finalize(core_outs, x, Sigma_inv, beta, gamma):
    """Combine per-core partial logits into the full softmax output."""
    x = np.asarray(x, dtype=np.float32)
    Sigma_inv = np.asarray(Sigma_inv, dtype=np.float32)
    beta = float(np.asarray(beta).reshape(-1)[0])
    gamma = float(np.asarray(gamma).reshape(-1)[0])

    partial = np.zeros((CP, N), dtype=np.float32)
    for o in core_outs:                                   # [16, N] each
        partial += o[:CP]
    partial = partial[:C]                                 # [C, N]
    x_sq = np.einsum("nd,d->n", x * x, Sigma_inv)         # [N]
    logits = np.exp(-beta * x_sq)[:, None].astype(np.float32) * partial.T
    z = gamma * logits
    z = z - z.max(axis=1, keepdims=True)
    ez = np.exp(z)
    return (ez / ez.sum(axis=1, keepdims=True)).astype(np.float32)


_NC_CACHE = {}


def kernel(x, exemplars, labels, Sigma_inv, beta, gamma):
    if "nc" not in _NC_CACHE:
        _NC_CACHE["nc"] = build_nc()
    nc = _NC_CACHE["nc"]
    in_maps = make_in_maps(x, exemplars, labels, Sigma_inv, beta, gamma)
    res = bass_utils.run_bass_kernel_spmd(nc, in_maps,
                                          core_ids=list(range(N_CORES)))
    core_outs = [r["out"] for r in res.results]
    return finalize(core_outs, x, Sigma_inv, beta, gamma)
